# revision 1
# baseline (speedup 1.0000x reference)
"""Trainium2 Bass kernel for nn_BiLSTMClassifier_4922032521432.

Strategy: 3 sequential phases (the bidirectional concat between stacked
BiLSTM layers is a hard barrier).  The big time-batched input GEMM for layer
1 runs tensor-parallel over all 8 cores (time-sliced).  Each recurrence phase
runs the two independent directions on 2 cores (one direction per core, the
backward direction on a host-reversed gate stream).  Layer-2/3 input GEMMs
are embedded in the recurrence kernels (SBUF-resident gate streams).  The
per-step cell update runs mostly on the Scalar (ACT) engine with
per-partition bias/scale access patterns; the cell-state update is a single
fused Vector-engine tensor_scalar op.  Layer-1 recurrent weights and hidden
state stream in bf16 (verified 2.6e-5 end-to-end error), halving the
dominant PE weight-load stream.
"""

import numpy as np
import ml_dtypes
import jax

import os as _os, tempfile as _tempfile
_cache = _os.environ.get("BASS_JAX_CACHE",
                         _os.path.join(_tempfile.gettempdir(), "bass_jax_cache"))
_os.makedirs(_cache, exist_ok=True)
jax.config.update("jax_compilation_cache_dir", _cache)
jax.config.update("jax_persistent_cache_min_entry_size_bytes", 0)
jax.config.update("jax_persistent_cache_min_compile_time_secs", 0)

import concourse.bass as bass
import concourse.bacc as bacc
import concourse.mybir as mybir
from concourse.tile import TileContext
from concourse.bass_utils import run_bass_kernel_spmd

fp32 = mybir.dt.float32
bf16 = mybir.dt.bfloat16
AF = mybir.ActivationFunctionType
ALU = mybir.AluOpType
ET = mybir.EngineType
ds = bass.ds

HINTS = (ET.PE, ET.Activation, ET.DVE)


def make_nc(n_cores):
    return bacc.Bacc("TRN2", target_bir_lowering=False, debug=False,
                     num_devices=n_cores)


# --------------------------------------------------------------------------
# gx1 GEMM kernel: gxT[G2, T_sl] = wT.T @ xT + b  (both directions stacked)
# --------------------------------------------------------------------------
def build_gx1(T_sl, In=1024, G2=2048):
    nc = make_nc(8)
    w = nc.declare_dram_parameter("wT", [In, G2], fp32, isOutput=False)
    b = nc.declare_dram_parameter("b2d", [128, G2 // 128], fp32, isOutput=False)
    xT = nc.declare_dram_parameter("xT", [In, T_sl], fp32, isOutput=False)
    out = nc.declare_dram_parameter("gxT", [G2, T_sl], fp32, isOutput=True)
    KT, GT = In // 128, G2 // 128
    Nt = min(512, T_sl)
    NT = T_sl // Nt
    with TileContext(nc) as tc:
        with (
            tc.tile_pool(name="const", bufs=1) as cp,
            tc.tile_pool(name="ps", bufs=4, space="PSUM") as pp,
            tc.tile_pool(name="ob", bufs=4) as op_,
        ):
            wsb = cp.tile([128, KT, G2], fp32)
            xsb = cp.tile([128, KT, T_sl], fp32)
            bsb = cp.tile([128, GT], fp32)
            for k in range(KT):
                nc.sync.dma_start(wsb[:, k, :], w[128 * k:128 * (k + 1), :])
                nc.sync.dma_start(xsb[:, k, :], xT[128 * k:128 * (k + 1), :])
            nc.sync.dma_start(bsb[:], b[:])
            for g in range(GT):
                for t in range(NT):
                    ps = pp.tile([128, Nt], fp32)
                    for k in range(KT):
                        nc.tensor.matmul(
                            ps[:], wsb[:, k, 128 * g:128 * (g + 1)],
                            xsb[:, k, Nt * t:Nt * (t + 1)],
                            start=(k == 0), stop=(k == KT - 1))
                    ob = op_.tile([128, Nt], fp32)
                    nc.vector.tensor_scalar_add(ob[:], ps[:], bsb[:, g:g + 1])
                    nc.sync.dma_start(
                        out[128 * g:128 * (g + 1), Nt * t:Nt * (t + 1)], ob[:])
    nc.compile()
    return nc


# --------------------------------------------------------------------------
# R1: H=256.  gates psum [128,6]+[128,2], h/c as [128,2]
# --------------------------------------------------------------------------
def build_r1(T, CH, wdt=fp32):
    nc = make_nc(2)
    C2 = 2 * CH
    Tpad = T + 2 * C2
    lhsT_d = nc.declare_dram_parameter("lhsT", [256, 1024], wdt, isOutput=False)
    gx_d = nc.declare_dram_parameter("gxT", [1024, Tpad], fp32, isOutput=False)
    y_d = nc.declare_dram_parameter("yT", [2, 128, T], fp32, isOutput=True)
    with TileContext(nc) as tc:
        with (
            tc.tile_pool(name="const", bufs=1) as cp,
            tc.tile_pool(name="psi", bufs=2, space="PSUM") as ppi,
            tc.tile_pool(name="psg", bufs=2, space="PSUM") as ppg,
            tc.tile_pool(name="wk", bufs=2) as wk,
        ):
            lhsT0 = cp.tile([128, 1024], wdt)
            lhsT1 = cp.tile([128, 1024], wdt)
            nc.sync.dma_start(lhsT0[:], lhsT_d[0:128, :])
            nc.sync.dma_start(lhsT1[:], lhsT_d[128:256, :])
            gxc = cp.tile([128, 8, C2], fp32)
            yr = cp.tile([128, 2, C2], fp32)
            hb = yr if wdt == fp32 else cp.tile([128, 2, C2], wdt)
            c0 = cp.tile([128, 2], fp32)
            c1 = cp.tile([128, 2], fp32)
            cbuf = [c0, c1]
            nc.vector.memset(c0[:], 0.0)
            nc.vector.memset(yr[:, :, C2 - 1:C2], 0.0)
            if hb is not yr:
                nc.vector.memset(hb[:, :, C2 - 1:C2], 0.0)
            for blk in range(8):
                nc.sync.dma_start(gxc[:, blk, 0:CH],
                                  gx_d[128 * blk:128 * (blk + 1), 0:CH])

            def step(s):
                hcol = (s - 1) % C2
                psi = ppi.tile([128, 6], fp32)
                psg = ppg.tile([128, 2], fp32)
                sifo = wk.tile([128, 6], fp32, tag="sifo")
                for c in range(6):
                    for k in range(2):
                        lt = (lhsT0, lhsT1)[k]
                        nc.tensor.matmul(psi[:, c:c + 1],
                                         lt[:, 128 * c:128 * (c + 1)],
                                         hb[:, k:k + 1, hcol:hcol + 1],
                                         start=(k == 0), stop=(k == 1))
                nc.vector.tensor_add(psi[:], psi[:], gxc[:, 0:6, s:s + 1])
                nc.scalar.activation(sifo[:], psi[:], AF.Sigmoid)
                for c in range(6, 8):
                    for k in range(2):
                        lt = (lhsT0, lhsT1)[k]
                        nc.tensor.matmul(psg[:, c - 6:c - 5],
                                         lt[:, 128 * c:128 * (c + 1)],
                                         hb[:, k:k + 1, hcol:hcol + 1],
                                         start=(k == 0), stop=(k == 1))
                nc.vector.tensor_add(psg[:], psg[:], gxc[:, 6:8, s:s + 1])
                gt = wk.tile([128, 2], fp32, tag="gt")
                nc.scalar.activation(gt[:], psg[:], AF.Tanh)
                m1 = wk.tile([128, 2], fp32, tag="m1")
                nc.vector.tensor_mul(m1[:], sifo[:, 0:2], gt[:])
                m2 = wk.tile([128, 2], fp32, tag="m2")
                nc.vector.tensor_mul(m2[:], sifo[:, 2:4], cbuf[s % 2][:])
                nc.vector.tensor_add(cbuf[(s + 1) % 2][:], m1[:], m2[:])
                tcc = wk.tile([128, 2], fp32, tag="tcc")
                nc.scalar.activation(tcc[:], cbuf[(s + 1) % 2][:], AF.Tanh)
                if hb is not yr:
                    nc.vector.tensor_mul(hb[:, :, s:s + 1], sifo[:, 4:6], tcc[:])
                nc.vector.tensor_mul(yr[:, :, s:s + 1], sifo[:, 4:6], tcc[:])

            with tc.For_i(0, T, C2, hint_engines=HINTS) as i:
                for blk in range(8):
                    nc.sync.dma_start(
                        gxc[:, blk, CH:C2],
                        gx_d[128 * blk:128 * (blk + 1), ds(i + CH, CH)])
                for s in range(CH):
                    step(s)
                for blk in range(8):
                    nc.sync.dma_start(
                        gxc[:, blk, 0:CH],
                        gx_d[128 * blk:128 * (blk + 1), ds(i + C2, CH)])
                for p in range(2):
                    nc.sync.dma_start(y_d[p, :, ds(i, CH)], yr[:, p:p + 1, 0:CH])
                for s in range(CH, C2):
                    step(s)
                for p in range(2):
                    nc.sync.dma_start(y_d[p, :, ds(i + CH, CH)],
                                      yr[:, p:p + 1, CH:C2])
    nc.compile()
    return nc


# --------------------------------------------------------------------------
# R2: H=64, embedded gx2 GEMM.  psum col A = [i;f], col B = [o;g]
# --------------------------------------------------------------------------
def build_r2(T, CH):
    nc = make_nc(2)
    C2 = 2 * CH
    y1_d = nc.declare_dram_parameter("y1T", [512, T], fp32, isOutput=False)
    w2_d = nc.declare_dram_parameter("w2ihT", [512, 256], fp32, isOutput=False)
    b2_d = nc.declare_dram_parameter("b2d", [128, 2], fp32, isOutput=False)
    l2_d = nc.declare_dram_parameter("lhsT2", [64, 256], fp32, isOutput=False)
    y2_d = nc.declare_dram_parameter("y2T", [64, T], fp32, isOutput=True)
    with TileContext(nc) as tc:
        with (
            tc.tile_pool(name="const", bufs=1) as cp,
            tc.tile_pool(name="rhs", bufs=8) as rp,
            tc.tile_pool(name="psG", bufs=4, space="PSUM") as ppG,
            tc.tile_pool(name="psA", bufs=2, space="PSUM") as ppA,
            tc.tile_pool(name="psB", bufs=2, space="PSUM") as ppB,
            tc.tile_pool(name="wk", bufs=2) as wk,
        ):
            w2sb = cp.tile([128, 4, 256], fp32)
            l2sb = cp.tile([64, 256], fp32)
            b2sb = cp.tile([128, 2], fp32)
            for k in range(4):
                nc.sync.dma_start(w2sb[:, k, :], w2_d[128 * k:128 * (k + 1), :])
            nc.sync.dma_start(l2sb[:], l2_d[:])
            nc.sync.dma_start(b2sb[:], b2_d[:])
            gxP = [cp.tile([128, T], fp32, tag=f"gx{p}", name=f"gxP{p}")
                   for p in range(2)]
            # --- embedded GEMM: gxP[p] = w2'[p-plane] @ y1T + b ---
            Nt = min(512, T)
            for tt in range(T // Nt):
                rhs = []
                for k in range(4):
                    r = rp.tile([128, Nt], fp32, tag=f"rhs{k}")
                    nc.sync.dma_start(
                        r[:], y1_d[128 * k:128 * (k + 1), Nt * tt:Nt * (tt + 1)])
                    rhs.append(r)
                for p in range(2):
                    ps = ppG.tile([128, Nt], fp32)
                    for k in range(4):
                        nc.tensor.matmul(ps[:], w2sb[:, k, 128 * p:128 * (p + 1)],
                                         rhs[k][:], start=(k == 0), stop=(k == 3))
                    nc.vector.tensor_scalar_add(
                        gxP[p][:, Nt * tt:Nt * (tt + 1)], ps[:],
                        b2sb[:, p:p + 1])
            # --- recurrence ---
            y2r = cp.tile([64, C2], fp32)
            gxc = cp.tile([128, 2, C2], fp32)
            c0 = cp.tile([64, 1], fp32)
            c1 = cp.tile([64, 1], fp32)
            cbuf = [c0, c1]
            nc.vector.memset(c0[:], 0.0)
            nc.vector.memset(y2r[:, C2 - 1:C2], 0.0)

            def step(s):
                hcol = (s - 1) % C2
                psA = ppA.tile([128, 1], fp32)
                psB = ppB.tile([128, 1], fp32)
                nc.tensor.matmul(psA[:], l2sb[:, 0:128], y2r[:, hcol:hcol + 1],
                                 start=True, stop=True)
                nc.tensor.matmul(psB[:], l2sb[:, 128:256], y2r[:, hcol:hcol + 1],
                                 start=True, stop=True)
                sc0 = wk.tile([128, 1], fp32, tag="sc0")   # [i'; f']
                nc.scalar.activation(sc0[:], psA[:], AF.Sigmoid,
                                     bias=gxc[:, 0:1, s:s + 1])
                gt = wk.tile([64, 1], fp32, tag="gt")
                nc.scalar.activation(gt[:], psB[64:128, :], AF.Tanh,
                                     bias=gxc[64:128, 1:2, s:s + 1])
                so = wk.tile([64, 1], fp32, tag="so")
                nc.scalar.activation(so[:], psB[0:64, :], AF.Sigmoid,
                                     bias=gxc[0:64, 1:2, s:s + 1])
                m1 = wk.tile([64, 1], fp32, tag="m1")
                nc.vector.tensor_mul(m1[:], sc0[0:64, :], gt[:])
                tcc = wk.tile([64, 1], fp32, tag="tcc")
                nc.scalar.activation(tcc[:], cbuf[s % 2][:], AF.Tanh,
                                     bias=m1[:], scale=sc0[64:128, :])
                nc.scalar.activation(y2r[:, s:s + 1], tcc[:], AF.Copy,
                                     scale=so[:])
                nc.vector.tensor_scalar(cbuf[(s + 1) % 2][:],
                                        cbuf[s % 2][:], sc0[64:128, :],
                                        m1[:], ALU.mult, ALU.add)

            with tc.For_i(0, T, C2, hint_engines=HINTS) as i:
                nc.vector.tensor_copy(gxc[:, 0:1, :], gxP[0][:, ds(i, C2)])
                nc.vector.tensor_copy(gxc[:, 1:2, :], gxP[1][:, ds(i, C2)])
                for s in range(C2):
                    step(s)
                nc.sync.dma_start(y2_d[:, ds(i, CH)], y2r[:, 0:CH])
                nc.sync.dma_start(y2_d[:, ds(i + CH, CH)], y2r[:, CH:C2])
    nc.compile()
    return nc


# --------------------------------------------------------------------------
# R3: H=32, embedded gx3 GEMM.  psum [128,1] = [i;f;o;g]
# --------------------------------------------------------------------------
def build_r3(T, CH):
    nc = make_nc(2)
    C2 = 2 * CH
    y2_d = nc.declare_dram_parameter("y2T", [128, T], fp32, isOutput=False)
    w3_d = nc.declare_dram_parameter("w3ihT", [128, 128], fp32, isOutput=False)
    b3_d = nc.declare_dram_parameter("b3d", [128, 1], fp32, isOutput=False)
    l3_d = nc.declare_dram_parameter("lhsT3", [32, 128], fp32, isOutput=False)
    h_d = nc.declare_dram_parameter("hout", [32, 1], fp32, isOutput=True)
    with TileContext(nc) as tc:
        with (
            tc.tile_pool(name="const", bufs=1) as cp,
            tc.tile_pool(name="psG", bufs=4, space="PSUM") as ppG,
            tc.tile_pool(name="psR", bufs=2, space="PSUM") as ppR,
            tc.tile_pool(name="wk", bufs=2) as wk,
        ):
            y2sb = cp.tile([128, T], fp32)
            w3sb = cp.tile([128, 128], fp32)
            l3sb = cp.tile([32, 128], fp32)
            b3sb = cp.tile([128, 1], fp32)
            nc.sync.dma_start(y2sb[:], y2_d[:])
            nc.sync.dma_start(w3sb[:], w3_d[:])
            nc.sync.dma_start(l3sb[:], l3_d[:])
            nc.sync.dma_start(b3sb[:], b3_d[:])
            gxP = cp.tile([128, T], fp32)
            Nt = min(512, T)
            for tt in range(T // Nt):
                ps = ppG.tile([128, Nt], fp32)
                nc.tensor.matmul(ps[:], w3sb[:],
                                 y2sb[:, Nt * tt:Nt * (tt + 1)],
                                 start=True, stop=True)
                nc.vector.tensor_scalar_add(gxP[:, Nt * tt:Nt * (tt + 1)],
                                            ps[:], b3sb[:, 0:1])
            h3 = cp.tile([32, 1], fp32)
            gxc = cp.tile([128, C2], fp32)
            c0 = cp.tile([32, 1], fp32)
            c1 = cp.tile([32, 1], fp32)
            cbuf = [c0, c1]
            nc.vector.memset(c0[:], 0.0)
            nc.vector.memset(h3[:], 0.0)

            def step(s):
                ps = ppR.tile([128, 1], fp32)
                nc.tensor.matmul(ps[:], l3sb[:], h3[:], start=True, stop=True)
                sifo = wk.tile([96, 1], fp32, tag="sifo")
                nc.scalar.activation(sifo[:], ps[0:96, :], AF.Sigmoid,
                                     bias=gxc[0:96, s:s + 1])
                gt = wk.tile([32, 1], fp32, tag="gt")
                nc.scalar.activation(gt[:], ps[96:128, :], AF.Tanh,
                                     bias=gxc[96:128, s:s + 1])
                m1 = wk.tile([32, 1], fp32, tag="m1")
                nc.scalar.activation(m1[:], gt[:], AF.Copy,
                                     scale=sifo[0:32, :])
                tcc = wk.tile([32, 1], fp32, tag="tcc")
                nc.scalar.activation(tcc[:], cbuf[s % 2][:], AF.Tanh,
                                     bias=m1[:], scale=sifo[32:64, :])
                nc.scalar.activation(h3[:], tcc[:], AF.Copy,
                                     scale=sifo[64:96, :])
                nc.vector.tensor_scalar(cbuf[(s + 1) % 2][:],
                                        cbuf[s % 2][:], sifo[32:64, :],
                                        m1[:], ALU.mult, ALU.add)

            with tc.For_i(0, T, C2, hint_engines=HINTS) as i:
                nc.vector.tensor_copy(gxc[:], gxP[:, ds(i, C2)])
                for s in range(C2):
                    step(s)
            nc.sync.dma_start(h_d[:], h3[:])
    nc.compile()
    return nc


# --------------------------------------------------------------------------
# Host-side prep + pipeline
# --------------------------------------------------------------------------
def perm_ifog(H):
    """pytorch gate rows [i,f,g,o] -> [i,f,o,g]"""
    return np.r_[0:2 * H, 3 * H:4 * H, 2 * H:3 * H]


def _c(a):
    return np.ascontiguousarray(a, dtype=np.float32)


class Pipeline:
    def __init__(self, T=8192, CH=64, r1_bf16=True):
        self.T, self.CH = T, CH
        self.r1_bf16 = r1_bf16
        self.nc_gx1 = build_gx1(T // 8)
        self.nc_r1 = build_r1(T, CH, wdt=bf16 if r1_bf16 else fp32)
        self.nc_r2 = build_r2(T, CH)
        self.nc_r3 = build_r3(T, CH)

    def __call__(self, inputs, timings=None):
        import time as _time
        T, CH = self.T, self.CH
        C2 = 2 * CH
        ii = dict(inputs)
        p1, p2, p3 = perm_ifog(256), perm_ifog(64), perm_ifog(32)

        def t(name, fn):
            t0 = _time.time()
            r = fn()
            if timings is not None:
                timings[name] = _time.time() - t0
            return r

        # ---- gx1 ----
        W1 = {}
        for d in "fb":
            W1[d] = dict(
                wih=ii[f"l1{d}_wih"][p1], whh=ii[f"l1{d}_whh"][p1],
                b=(ii[f"l1{d}_bih"] + ii[f"l1{d}_bhh"])[p1])
        wT = _c(np.concatenate([W1["f"]["wih"], W1["b"]["wih"]], 0).T)
        bb = np.concatenate([W1["f"]["b"], W1["b"]["b"]])
        b2d = _c(bb.reshape(16, 128).T)
        xT = _c(ii["x"].T)
        sl = T // 8
        maps = [{"wT": wT, "b2d": b2d, "xT": _c(xT[:, r * sl:(r + 1) * sl])}
                for r in range(8)]
        res = t("gx1", lambda: run_bass_kernel_spmd(
            self.nc_gx1, maps, list(range(8))).results)
        gx1 = np.concatenate([r["gxT"] for r in res], axis=1)  # [2048, T]
        pad = np.zeros((1024, 2 * C2), np.float32)
        gx1f = _c(np.concatenate([gx1[:1024], pad], 1))
        gx1b = _c(np.concatenate([gx1[1024:][:, ::-1], pad], 1))

        # ---- R1 ----
        wdt_np = ml_dtypes.bfloat16 if self.r1_bf16 else np.float32
        maps = [
            {"lhsT": np.ascontiguousarray(W1["f"]["whh"].T).astype(wdt_np),
             "gxT": gx1f},
            {"lhsT": np.ascontiguousarray(W1["b"]["whh"].T).astype(wdt_np),
             "gxT": gx1b},
        ]
        res = t("r1", lambda: run_bass_kernel_spmd(
            self.nc_r1, maps, [0, 1]).results)
        y1f = res[0]["yT"].reshape(256, T)
        y1b = res[1]["yT"].reshape(256, T)[:, ::-1]
        y1T = np.concatenate([y1f, y1b], 0)          # [512, T]

        # ---- R2 ----
        W2 = {}
        for d in "fb":
            W2[d] = dict(
                wih=ii[f"l2{d}_wih"][p2], whh=ii[f"l2{d}_whh"][p2],
                b=(ii[f"l2{d}_bih"] + ii[f"l2{d}_bhh"])[p2])
        maps = []
        for d, yy in (("f", y1T), ("b", y1T[:, ::-1])):
            maps.append({
                "y1T": _c(yy),
                "w2ihT": _c(W2[d]["wih"].T),
                "b2d": _c(W2[d]["b"].reshape(2, 128).T),
                "lhsT2": _c(W2[d]["whh"].T),
            })
        res = t("r2", lambda: run_bass_kernel_spmd(
            self.nc_r2, maps, [0, 1]).results)
        y2f = res[0]["y2T"]
        y2b = res[1]["y2T"][:, ::-1]
        y2T = np.concatenate([y2f, y2b], 0)          # [128, T]

        # ---- R3 ----
        W3 = {}
        for d in "fb":
            W3[d] = dict(
                wih=ii[f"l3{d}_wih"][p3], whh=ii[f"l3{d}_whh"][p3],
                b=(ii[f"l3{d}_bih"] + ii[f"l3{d}_bhh"])[p3])
        maps = []
        for d, yy in (("f", y2T), ("b", y2T[:, ::-1])):
            maps.append({
                "y2T": _c(yy),
                "w3ihT": _c(W3[d]["wih"].T),
                "b3d": _c(W3[d]["b"].reshape(128, 1)),
                "lhsT3": _c(W3[d]["whh"].T),
            })
        res = t("r3", lambda: run_bass_kernel_spmd(
            self.nc_r3, maps, [0, 1]).results)
        h3f = res[0]["hout"][:, 0]
        h3b = res[1]["hout"][:, 0]

        # ---- head (host; 1.2 KFLOP) ----
        feat = np.concatenate([h3f, h3b])[None, :]
        z = feat @ ii["w1"].T + ii["b1"]
        z = z @ ii["w2"].T + ii["b2"]
        return z.astype(np.float32)


# --------------------------------------------------------------------------
# harness entry point
# --------------------------------------------------------------------------
_PIPE = None


def kernel(**inputs):
    global _PIPE
    if _PIPE is None:
        _PIPE = Pipeline(T=8192, CH=64, r1_bf16=True)
    inp = {k: np.asarray(v) for k, v in inputs.items()}
    return _PIPE(inp)



# revision 7
# speedup vs baseline: 8.0400x; 8.0400x over previous
"""Trainium2 Bass kernel for nn_BiLSTMClassifier_4922032521432.

Single-launch fused pipeline on 2 NeuronCores (SPMD, identical code; all
direction differences are expressed as per-core data).  Core 0 runs the
forward direction of every layer, core 1 the backward direction — each in
its own time order, so the recurrence code is direction-agnostic.

Data movement strategy (the previous 4-launch version shipped ~380MB per
call through the axon tunnel at ~90MB/s; this ships ~22MB once):
  - x is shipped once, split in time halves (one per core), bf16, transposed
    on host to [1024, 4096] per core.
  - Each core writes its half forward + DVE-reversed into a contribution
    buffer; a DRAM AllGather gives both cores both halves in both orders.
    Core c reads half h of its own stream at gathered[(h XOR c), c] via
    partition-id-affine dynamic DMA offsets.
  - Layer-1 gates (gx1) stay SBUF-resident in bf16 (16.5MB).
  - y1/y2 cross over between cores as bf16 AllGathers, with the reversed
    copies produced in-loop by DVE negative-stride copies (full speed,
    unlike negative-stride DMA which is ~5x slow).
  - Output is just the final [32] hidden vector per core; the 1.2KFLOP
    classifier head runs on host.
"""

import numpy as np
import ml_dtypes
import jax

import os as _os, tempfile as _tempfile
_cache = _os.environ.get("BASS_JAX_CACHE",
                         _os.path.join(_tempfile.gettempdir(), "bass_jax_cache"))
_os.makedirs(_cache, exist_ok=True)
jax.config.update("jax_compilation_cache_dir", _cache)
jax.config.update("jax_persistent_cache_min_entry_size_bytes", 0)
jax.config.update("jax_persistent_cache_min_compile_time_secs", 0)

import concourse.bass as bass
import concourse.bacc as bacc
import concourse.mybir as mybir
from concourse.tile import TileContext
from concourse.bass_utils import run_bass_kernel_spmd

fp32 = mybir.dt.float32
bf16 = mybir.dt.bfloat16
AF = mybir.ActivationFunctionType
ALU = mybir.AluOpType
ET = mybir.EngineType
ds = bass.ds

HINTS = (ET.PE, ET.Activation, ET.DVE)

T = 8192
Th = T // 2
CH = 64
C2 = 2 * CH
Nt = 512
NB = T // Nt
TPAD = T + 2 * C2


def build_fused():
    nc = bacc.Bacc("TRN2", target_bir_lowering=False, debug=False,
                   num_devices=2)
    # ---- per-core parameters (direction-specific data) ----
    xTh = nc.declare_dram_parameter("xTh", [1024, Th], bf16, isOutput=False)
    w1T = nc.declare_dram_parameter("w1T", [1024, 1024], bf16, isOutput=False)
    b1d = nc.declare_dram_parameter("b1d", [128, 8], fp32, isOutput=False)
    l1T = nc.declare_dram_parameter("l1T", [256, 1024], bf16, isOutput=False)
    w2T = nc.declare_dram_parameter("w2T", [512, 256], bf16, isOutput=False)
    b2d = nc.declare_dram_parameter("b2d", [128, 2], fp32, isOutput=False)
    l2T = nc.declare_dram_parameter("l2T", [64, 256], fp32, isOutput=False)
    w3T = nc.declare_dram_parameter("w3T", [128, 128], bf16, isOutput=False)
    b3d = nc.declare_dram_parameter("b3d", [128, 1], fp32, isOutput=False)
    l3T = nc.declare_dram_parameter("l3T", [32, 128], fp32, isOutput=False)
    hout = nc.declare_dram_parameter("hout", [32, 1], fp32, isOutput=True)
    # ---- internal DRAM ----
    xcb = nc.dram_tensor("xcb", [2, 8, 128, Th], bf16)
    XG = nc.dram_tensor("XG", [2, 2, 8, 128, Th], bf16)
    y1cb = nc.dram_tensor("y1cb", [2, 2, 128, T], bf16)
    Y1G = nc.dram_tensor("Y1G", [2, 2, 2, 128, T], bf16)
    y2cb = nc.dram_tensor("y2cb", [2, 64, T], bf16)
    Y2G = nc.dram_tensor("Y2G", [2, 2, 64, T], bf16)

    with TileContext(nc) as tc:
        myid = nc.partition_id()
        oth = 1 - myid

        # ================= P0: x contributions + gather =================
        with tc.tile_pool(name="p0", bufs=3) as p0:
            for k in range(8):
                t = p0.tile([128, Th], bf16, tag="t")
                r = p0.tile([128, Th], bf16, tag="r")
                nc.sync.dma_start(t[:], xTh[128 * k:128 * (k + 1), :])
                nc.gpsimd.dma_start(xcb[0, k], t[:])
                nc.vector.tensor_copy(r[:], t[:, ::-1])
                nc.scalar.dma_start(xcb[1, k], r[:])
            nc.gpsimd.collective_compute(
                "AllGather", mybir.AluOpType.bypass,
                replica_groups=[[0, 1]],
                ins=[xcb[:].opt()], outs=[XG[:].opt()])

        # ============ P1 + R1 (gx1 SBUF-resident, then recurrence) ======
        with tc.tile_pool(name="gx1glob", bufs=1) as gp:
            gx1sb = gp.tile([128, 8, TPAD], bf16)

            # ---- P1: layer-1 input GEMM into gx1sb ----
            with (
                tc.tile_pool(name="p1c", bufs=1) as p1c,
                tc.tile_pool(name="p1r", bufs=2) as p1r,
                tc.tile_pool(name="ps1", bufs=4, space="PSUM") as pp1,
            ):
                w1sb = p1c.tile([128, 8, 1024], bf16)
                b1sb = p1c.tile([128, 8], fp32)
                for k in range(8):
                    nc.sync.dma_start(w1sb[:, k, :],
                                      w1T[128 * k:128 * (k + 1), :])
                nc.sync.dma_start(b1sb[:], b1d[:])
                for b in range(NB):
                    h = (b * Nt) // Th
                    src = myid if h == 0 else oth
                    c0 = (b * Nt) % Th
                    rhs = p1r.tile([128, 8, Nt], bf16, tag="rhs")
                    for k in range(8):
                        eng = (nc.sync, nc.gpsimd, nc.scalar)[k % 3]
                        eng.dma_start(
                            rhs[:, k, :],
                            XG[ds(src, 1), ds(myid, 1), k, :, c0:c0 + Nt].opt())
                    for g in range(8):
                        ps = pp1.tile([128, Nt], fp32)
                        for k in range(8):
                            nc.tensor.matmul(ps[:],
                                             w1sb[:, k, 128 * g:128 * (g + 1)],
                                             rhs[:, k, :],
                                             start=(k == 0), stop=(k == 7))
                        nc.vector.tensor_scalar_add(
                            gx1sb[:, g, b * Nt:(b + 1) * Nt], ps[:],
                            b1sb[:, g:g + 1])

            # ---- R1: H=256 recurrence ----
            with (
                tc.tile_pool(name="r1c", bufs=1) as cp,
                tc.tile_pool(name="psi", bufs=2, space="PSUM") as ppi,
                tc.tile_pool(name="psg", bufs=2, space="PSUM") as ppg,
                tc.tile_pool(name="r1w", bufs=2) as wk,
            ):
                lhsT0 = cp.tile([128, 1024], bf16)
                lhsT1 = cp.tile([128, 1024], bf16)
                nc.sync.dma_start(lhsT0[:], l1T[0:128, :])
                nc.sync.dma_start(lhsT1[:], l1T[128:256, :])
                gxc = cp.tile([128, 8, C2], fp32)
                hb = cp.tile([128, 2, C2], bf16)
                c0t = cp.tile([128, 2], fp32)
                c1t = cp.tile([128, 2], fp32)
                cbuf = [c0t, c1t]
                nc.vector.memset(c0t[:], 0.0)
                nc.vector.memset(hb[:, :, C2 - 1:C2], 0.0)
                nc.vector.tensor_copy(gxc[:, :, 0:CH], gx1sb[:, :, 0:CH])

                def step1(s):
                    hcol = (s - 1) % C2
                    psi = ppi.tile([128, 6], fp32)
                    psg = ppg.tile([128, 2], fp32)
                    sifo = wk.tile([128, 6], fp32, tag="sifo")
                    for c in range(6):
                        for k in range(2):
                            lt = (lhsT0, lhsT1)[k]
                            nc.tensor.matmul(psi[:, c:c + 1],
                                             lt[:, 128 * c:128 * (c + 1)],
                                             hb[:, k:k + 1, hcol:hcol + 1],
                                             start=(k == 0), stop=(k == 1))
                    nc.vector.tensor_add(psi[:], psi[:], gxc[:, 0:6, s:s + 1])
                    nc.scalar.activation(sifo[:], psi[:], AF.Sigmoid)
                    for c in range(6, 8):
                        for k in range(2):
                            lt = (lhsT0, lhsT1)[k]
                            nc.tensor.matmul(psg[:, c - 6:c - 5],
                                             lt[:, 128 * c:128 * (c + 1)],
                                             hb[:, k:k + 1, hcol:hcol + 1],
                                             start=(k == 0), stop=(k == 1))
                    nc.vector.tensor_add(psg[:], psg[:], gxc[:, 6:8, s:s + 1])
                    gt = wk.tile([128, 2], fp32, tag="gt")
                    nc.scalar.activation(gt[:], psg[:], AF.Tanh)
                    m1 = wk.tile([128, 2], fp32, tag="m1")
                    nc.vector.tensor_mul(m1[:], sifo[:, 0:2], gt[:])
                    m2 = wk.tile([128, 2], fp32, tag="m2")
                    nc.vector.tensor_mul(m2[:], sifo[:, 2:4], cbuf[s % 2][:])
                    nc.vector.tensor_add(cbuf[(s + 1) % 2][:], m1[:], m2[:])
                    tcc = wk.tile([128, 2], fp32, tag="tcc")
                    nc.scalar.activation(tcc[:], cbuf[(s + 1) % 2][:], AF.Tanh)
                    nc.vector.tensor_mul(hb[:, :, s:s + 1], sifo[:, 4:6],
                                         tcc[:])

                def wr1(i, lo):
                    # write hb[:, :, lo:lo+CH] fwd + reversed
                    for p in range(2):
                        nc.sync.dma_start(y1cb[0, p, :, ds(i + lo, CH)],
                                          hb[:, p, lo:lo + CH])
                        hr = wk.tile([128, CH], bf16, tag=f"hr{p}")
                        nc.vector.tensor_copy(hr[:],
                                              hb[:, p, lo:lo + CH][:, ::-1])
                        nc.gpsimd.dma_start(
                            y1cb[1, p, :, ds(T - CH - lo - i, CH)], hr[:])

                with tc.For_i(0, T, C2, hint_engines=HINTS) as i:
                    nc.vector.tensor_copy(gxc[:, :, CH:C2],
                                          gx1sb[:, :, ds(i + CH, CH)])
                    for s in range(CH):
                        step1(s)
                    nc.vector.tensor_copy(gxc[:, :, 0:CH],
                                          gx1sb[:, :, ds(i + C2, CH)])
                    wr1(i, 0)
                    for s in range(CH, C2):
                        step1(s)
                    wr1(i, CH)

        nc.gpsimd.collective_compute(
            "AllGather", mybir.AluOpType.bypass,
            replica_groups=[[0, 1]],
            ins=[y1cb[:].opt()], outs=[Y1G[:].opt()])

        # ================= R2: H=64, embedded gx2 GEMM ==================
        with (
            tc.tile_pool(name="r2c", bufs=1) as cp,
            tc.tile_pool(name="r2r", bufs=4) as rp,
            tc.tile_pool(name="psG", bufs=4, space="PSUM") as ppG,
            tc.tile_pool(name="psA", bufs=2, space="PSUM") as ppA,
            tc.tile_pool(name="psB", bufs=2, space="PSUM") as ppB,
            tc.tile_pool(name="r2w", bufs=2) as wk,
        ):
            w2sb = cp.tile([128, 4, 256], bf16)
            l2sb = cp.tile([64, 256], fp32)
            b2sb = cp.tile([128, 2], fp32)
            for k in range(4):
                nc.sync.dma_start(w2sb[:, k, :], w2T[128 * k:128 * (k + 1), :])
            nc.sync.dma_start(l2sb[:], l2T[:])
            nc.sync.dma_start(b2sb[:], b2d[:])
            gxP = [cp.tile([128, T], fp32, tag=f"gx{p}", name=f"gxP{p}")
                   for p in range(2)]
            for b in range(NB):
                rhs = rp.tile([128, 4, Nt], bf16, tag="rhs")
                for k in range(2):
                    nc.sync.dma_start(
                        rhs[:, k, :],
                        Y1G[ds(myid, 1), 0, k, :, b * Nt:(b + 1) * Nt].opt())
                    nc.gpsimd.dma_start(
                        rhs[:, 2 + k, :],
                        Y1G[ds(oth, 1), 1, k, :, b * Nt:(b + 1) * Nt].opt())
                for p in range(2):
                    ps = ppG.tile([128, Nt], fp32)
                    for k in range(4):
                        nc.tensor.matmul(ps[:], w2sb[:, k, 128 * p:128 * (p + 1)],
                                         rhs[:, k, :], start=(k == 0),
                                         stop=(k == 3))
                    nc.vector.tensor_scalar_add(
                        gxP[p][:, b * Nt:(b + 1) * Nt], ps[:], b2sb[:, p:p + 1])
            # ---- recurrence ----
            y2r = cp.tile([64, C2], fp32)
            gxc = cp.tile([128, 2, C2], fp32)
            c0t = cp.tile([64, 1], fp32)
            c1t = cp.tile([64, 1], fp32)
            cbuf = [c0t, c1t]
            nc.vector.memset(c0t[:], 0.0)
            nc.vector.memset(y2r[:, C2 - 1:C2], 0.0)

            def step2(s):
                hcol = (s - 1) % C2
                psA = ppA.tile([128, 1], fp32)
                psB = ppB.tile([128, 1], fp32)
                nc.tensor.matmul(psA[:], l2sb[:, 0:128], y2r[:, hcol:hcol + 1],
                                 start=True, stop=True)
                nc.tensor.matmul(psB[:], l2sb[:, 128:256], y2r[:, hcol:hcol + 1],
                                 start=True, stop=True)
                sc0 = wk.tile([128, 1], fp32, tag="sc0")   # [i'; f']
                nc.scalar.activation(sc0[:], psA[:], AF.Sigmoid,
                                     bias=gxc[:, 0:1, s:s + 1])
                gt = wk.tile([64, 1], fp32, tag="gt")
                nc.scalar.activation(gt[:], psB[64:128, :], AF.Tanh,
                                     bias=gxc[64:128, 1:2, s:s + 1])
                so = wk.tile([64, 1], fp32, tag="so")
                nc.scalar.activation(so[:], psB[0:64, :], AF.Sigmoid,
                                     bias=gxc[0:64, 1:2, s:s + 1])
                m1 = wk.tile([64, 1], fp32, tag="m1")
                nc.vector.tensor_mul(m1[:], sc0[0:64, :], gt[:])
                tcc = wk.tile([64, 1], fp32, tag="tcc")
                nc.scalar.activation(tcc[:], cbuf[s % 2][:], AF.Tanh,
                                     bias=m1[:], scale=sc0[64:128, :])
                nc.scalar.activation(y2r[:, s:s + 1], tcc[:], AF.Copy,
                                     scale=so[:])
                nc.vector.tensor_scalar(cbuf[(s + 1) % 2][:],
                                        cbuf[s % 2][:], sc0[64:128, :],
                                        m1[:], ALU.mult, ALU.add)

            def wr2(i, lo):
                y16 = wk.tile([64, CH], bf16, tag="y16")
                nc.vector.tensor_copy(y16[:], y2r[:, lo:lo + CH])
                nc.sync.dma_start(y2cb[0, :, ds(i + lo, CH)], y16[:])
                y16r = wk.tile([64, CH], bf16, tag="y16r")
                nc.vector.tensor_copy(y16r[:], y2r[:, lo:lo + CH][:, ::-1])
                nc.gpsimd.dma_start(y2cb[1, :, ds(T - CH - lo - i, CH)], y16r[:])

            with tc.For_i(0, T, C2, hint_engines=HINTS) as i:
                nc.vector.tensor_copy(gxc[:, 0:1, :], gxP[0][:, ds(i, C2)])
                nc.vector.tensor_copy(gxc[:, 1:2, :], gxP[1][:, ds(i, C2)])
                for s in range(CH):
                    step2(s)
                wr2(i, 0)
                for s in range(CH, C2):
                    step2(s)
                wr2(i, CH)

        nc.gpsimd.collective_compute(
            "AllGather", mybir.AluOpType.bypass,
            replica_groups=[[0, 1]],
            ins=[y2cb[:].opt()], outs=[Y2G[:].opt()])

        # ================= R3: H=32, embedded gx3 GEMM ==================
        with (
            tc.tile_pool(name="r3c", bufs=1) as cp,
            tc.tile_pool(name="r3r", bufs=4) as rp,
            tc.tile_pool(name="ps3G", bufs=4, space="PSUM") as ppG,
            tc.tile_pool(name="ps3R", bufs=2, space="PSUM") as ppR,
            tc.tile_pool(name="r3w", bufs=2) as wk,
        ):
            w3sb = cp.tile([128, 128], bf16)
            l3sb = cp.tile([32, 128], fp32)
            b3sb = cp.tile([128, 1], fp32)
            nc.sync.dma_start(w3sb[:], w3T[:])
            nc.sync.dma_start(l3sb[:], l3T[:])
            nc.sync.dma_start(b3sb[:], b3d[:])
            gx3 = cp.tile([128, T], fp32)
            for b in range(NB):
                rhs = rp.tile([128, Nt], bf16, tag="rhs")
                nc.sync.dma_start(
                    rhs[0:64, :],
                    Y2G[ds(myid, 1), 0, :, b * Nt:(b + 1) * Nt].opt())
                nc.gpsimd.dma_start(
                    rhs[64:128, :],
                    Y2G[ds(oth, 1), 1, :, b * Nt:(b + 1) * Nt].opt())
                ps = ppG.tile([128, Nt], fp32)
                nc.tensor.matmul(ps[:], w3sb[:], rhs[:], start=True, stop=True)
                nc.vector.tensor_scalar_add(gx3[:, b * Nt:(b + 1) * Nt],
                                            ps[:], b3sb[:, 0:1])
            h3 = cp.tile([32, 1], fp32)
            gxc = cp.tile([128, C2], fp32)
            c0t = cp.tile([32, 1], fp32)
            c1t = cp.tile([32, 1], fp32)
            cbuf = [c0t, c1t]
            nc.vector.memset(c0t[:], 0.0)
            nc.vector.memset(h3[:], 0.0)

            def step3(s):
                ps = ppR.tile([128, 1], fp32)
                nc.tensor.matmul(ps[:], l3sb[:], h3[:], start=True, stop=True)
                sifo = wk.tile([96, 1], fp32, tag="sifo")
                nc.scalar.activation(sifo[:], ps[0:96, :], AF.Sigmoid,
                                     bias=gxc[0:96, s:s + 1])
                gt = wk.tile([32, 1], fp32, tag="gt")
                nc.scalar.activation(gt[:], ps[96:128, :], AF.Tanh,
                                     bias=gxc[96:128, s:s + 1])
                m1 = wk.tile([32, 1], fp32, tag="m1")
                nc.scalar.activation(m1[:], gt[:], AF.Copy,
                                     scale=sifo[0:32, :])
                tcc = wk.tile([32, 1], fp32, tag="tcc")
                nc.scalar.activation(tcc[:], cbuf[s % 2][:], AF.Tanh,
                                     bias=m1[:], scale=sifo[32:64, :])
                nc.scalar.activation(h3[:], tcc[:], AF.Copy,
                                     scale=sifo[64:96, :])
                nc.vector.tensor_scalar(cbuf[(s + 1) % 2][:],
                                        cbuf[s % 2][:], sifo[32:64, :],
                                        m1[:], ALU.mult, ALU.add)

            with tc.For_i(0, T, C2, hint_engines=HINTS) as i:
                nc.vector.tensor_copy(gxc[:], gx3[:, ds(i, C2)])
                for s in range(C2):
                    step3(s)
            nc.sync.dma_start(hout[:], h3[:])
    nc.compile()
    return nc


# --------------------------------------------------------------------------
# Host-side prep + launch
# --------------------------------------------------------------------------
def perm_ifog(H):
    """pytorch gate rows [i,f,g,o] -> [i,f,o,g]"""
    return np.r_[0:2 * H, 3 * H:4 * H, 2 * H:3 * H]


def _c32(a):
    return np.ascontiguousarray(a, dtype=np.float32)


def _c16(a):
    return np.ascontiguousarray(np.asarray(a).astype(ml_dtypes.bfloat16))


class Pipeline:
    def __init__(self, **kw):
        self.nc = build_fused()

    def __call__(self, inputs, timings=None):
        import time as _time
        ii = {k: np.asarray(v) for k, v in inputs.items()}
        p1, p2, p3 = perm_ifog(256), perm_ifog(64), perm_ifog(32)
        xx = ii["x"].astype(ml_dtypes.bfloat16)   # [8192, 1024]
        maps = []
        for c, d in ((0, "f"), (1, "b")):
            b1 = (ii[f"l1{d}_bih"] + ii[f"l1{d}_bhh"])[p1]
            b2 = (ii[f"l2{d}_bih"] + ii[f"l2{d}_bhh"])[p2]
            b3 = (ii[f"l3{d}_bih"] + ii[f"l3{d}_bhh"])[p3]
            # The rhs in the layer-2/3 GEMMs is [own-direction stream;
            # other-direction stream].  For the backward core that order is
            # (y_b, y_f), so swap the input-row halves of its W_ih.
            w2Trows = ii[f"l2{d}_wih"][p2].T     # [512, 256]
            w3Trows = ii[f"l3{d}_wih"][p3].T     # [128, 128]
            if c == 1:
                w2Trows = np.concatenate([w2Trows[256:], w2Trows[:256]], 0)
                w3Trows = np.concatenate([w3Trows[64:], w3Trows[:64]], 0)
            maps.append({
                "xTh": np.ascontiguousarray(xx[c * Th:(c + 1) * Th].T),
                "w1T": _c16(ii[f"l1{d}_wih"][p1].T),
                "b1d": _c32(b1.reshape(8, 128).T),
                "l1T": _c16(ii[f"l1{d}_whh"][p1].T),
                "w2T": _c16(w2Trows),
                "b2d": _c32(b2.reshape(2, 128).T),
                "l2T": _c32(ii[f"l2{d}_whh"][p2].T),
                "w3T": _c16(w3Trows),
                "b3d": _c32(b3.reshape(128, 1)),
                "l3T": _c32(ii[f"l3{d}_whh"][p3].T),
            })
        t0 = _time.time()
        res = run_bass_kernel_spmd(self.nc, maps, [0, 1]).results
        if timings is not None:
            timings["fused"] = _time.time() - t0
        h3f = res[0]["hout"][:, 0]
        h3b = res[1]["hout"][:, 0]
        feat = np.concatenate([h3f, h3b])[None, :].astype(np.float32)
        z = feat @ ii["w1"].T + ii["b1"]
        z = z @ ii["w2"].T + ii["b2"]
        return z.astype(np.float32)


# --------------------------------------------------------------------------
# harness entry point
# --------------------------------------------------------------------------
_PIPE = None


def kernel(**inputs):
    global _PIPE
    if _PIPE is None:
        _PIPE = Pipeline()
    inp = {k: np.asarray(v) for k, v in inputs.items()}
    return _PIPE(inp)


# revision 12
# speedup vs baseline: 11.3314x; 1.4094x over previous
"""Trainium2 Bass kernel for nn_BiLSTMClassifier_4922032521432.

Single-launch fused pipeline on 2 NeuronCores (SPMD, identical code; all
direction differences are expressed as per-core data).  Core 0 runs the
forward direction of every layer, core 1 the backward direction — each in
its own time order, so the recurrence code is direction-agnostic.

Data movement strategy (the previous 4-launch version shipped ~380MB per
call through the axon tunnel at ~90MB/s; this ships ~22MB once):
  - x is shipped once, split in time halves (one per core), bf16, transposed
    on host to [1024, 4096] per core.
  - Each core writes its half forward + DVE-reversed into a contribution
    buffer; a DRAM AllGather gives both cores both halves in both orders.
    Core c reads half h of its own stream at gathered[(h XOR c), c] via
    partition-id-affine dynamic DMA offsets.
  - Layer-1 gates (gx1) stay SBUF-resident in bf16 (16.5MB).
  - y1/y2 cross over between cores as bf16 AllGathers, with the reversed
    copies produced in-loop by DVE negative-stride copies (full speed,
    unlike negative-stride DMA which is ~5x slow).
  - Output is just the final [32] hidden vector per core; the 1.2KFLOP
    classifier head runs on host.
"""

import numpy as np
import ml_dtypes
import jax

import os as _os, tempfile as _tempfile
_cache = _os.environ.get("BASS_JAX_CACHE",
                         _os.path.join(_tempfile.gettempdir(), "bass_jax_cache"))
_os.makedirs(_cache, exist_ok=True)
jax.config.update("jax_compilation_cache_dir", _cache)
jax.config.update("jax_persistent_cache_min_entry_size_bytes", 0)
jax.config.update("jax_persistent_cache_min_compile_time_secs", 0)

import concourse.bass as bass
import concourse.bacc as bacc
import concourse.mybir as mybir
from concourse.tile import TileContext
from concourse.bass_utils import run_bass_kernel_spmd

fp32 = mybir.dt.float32
bf16 = mybir.dt.bfloat16
fp8 = mybir.dt.float8e4
XDT = fp8                      # dtype for x and the layer-1 GEMM operands
XDT_NP = "float8_e4m3"         # ml_dtypes name matching XDT
AF = mybir.ActivationFunctionType
ALU = mybir.AluOpType
ET = mybir.EngineType
ds = bass.ds

HINTS = (ET.PE, ET.Activation, ET.DVE)

T = 8192
Th = T // 2
CH = 64
C2 = 2 * CH
Nt = 512
NB = T // Nt
TPAD = T + 2 * C2


def build_fused(nphases=4):
    nc = bacc.Bacc("TRN2", target_bir_lowering=False, debug=False,
                   num_devices=2)
    # ---- per-core parameters (direction-specific data) ----
    xTh = nc.declare_dram_parameter("xTh", [1024, Th], XDT, isOutput=False)
    w1T = nc.declare_dram_parameter("w1T", [1024, 1024], XDT, isOutput=False)
    b1d = nc.declare_dram_parameter("b1d", [128, 8], fp32, isOutput=False)
    l1T = nc.declare_dram_parameter("l1T", [256, 1024], bf16, isOutput=False)
    w2T = nc.declare_dram_parameter("w2T", [512, 256], bf16, isOutput=False)
    b2d = nc.declare_dram_parameter("b2d", [128, 2], fp32, isOutput=False)
    l2T = nc.declare_dram_parameter("l2T", [64, 256], fp32, isOutput=False)
    w3T = nc.declare_dram_parameter("w3T", [128, 128], bf16, isOutput=False)
    b3d = nc.declare_dram_parameter("b3d", [128, 1], fp32, isOutput=False)
    l3T = nc.declare_dram_parameter("l3T", [32, 128], fp32, isOutput=False)
    hout = nc.declare_dram_parameter("hout", [32, 1], fp32, isOutput=True)
    # ---- internal DRAM ----
    xcb = nc.dram_tensor("xcb", [2, 8, 128, Th], XDT)
    XG = nc.dram_tensor("XG", [2, 2, 8, 128, Th], XDT)
    y1cb = nc.dram_tensor("y1cb", [2, 2, 128, T], bf16)
    Y1G = nc.dram_tensor("Y1G", [2, 2, 2, 128, T], bf16)
    y2cb = nc.dram_tensor("y2cb", [2, 64, T], bf16)
    Y2G = nc.dram_tensor("Y2G", [2, 2, 64, T], bf16)

    with TileContext(nc) as tc:
        myid = nc.partition_id()
        oth = 1 - myid

        # ================= P0: x contributions + gather =================
        with tc.tile_pool(name="p0", bufs=3) as p0:
            for k in range(8):
                t = p0.tile([128, Th], XDT, tag="t")
                r = p0.tile([128, Th], XDT, tag="r")
                nc.sync.dma_start(t[:], xTh[128 * k:128 * (k + 1), :])
                nc.gpsimd.dma_start(xcb[0, k], t[:])
                nc.vector.tensor_copy(r[:], t[:, ::-1])
                nc.scalar.dma_start(xcb[1, k], r[:])
            nc.gpsimd.collective_compute(
                "AllGather", mybir.AluOpType.bypass,
                replica_groups=[[0, 1]],
                ins=[xcb[:].opt()], outs=[XG[:].opt()])

        # ============ P1 + R1 (gx1 SBUF-resident, then recurrence) ======
        if nphases >= 2:
         with tc.tile_pool(name="gx1glob", bufs=1) as gp:
            gx1sb = gp.tile([128, 8, TPAD], bf16)

            # ---- P1: layer-1 input GEMM into gx1sb ----
            with (
                tc.tile_pool(name="p1c", bufs=1) as p1c,
                tc.tile_pool(name="p1r", bufs=2) as p1r,
                tc.tile_pool(name="ps1", bufs=4, space="PSUM") as pp1,
            ):
                w1sb = p1c.tile([128, 8, 1024], XDT)
                b1sb = p1c.tile([128, 8], fp32)
                for k in range(8):
                    nc.sync.dma_start(w1sb[:, k, :],
                                      w1T[128 * k:128 * (k + 1), :])
                nc.sync.dma_start(b1sb[:], b1d[:])
                for b in range(NB):
                    h = (b * Nt) // Th
                    src = myid if h == 0 else oth
                    c0 = (b * Nt) % Th
                    rhs = p1r.tile([128, 8, Nt], XDT, tag="rhs")
                    for k in range(8):
                        eng = (nc.sync, nc.gpsimd, nc.scalar)[k % 3]
                        eng.dma_start(
                            rhs[:, k, :],
                            XG[ds(src, 1), ds(myid, 1), k, :, c0:c0 + Nt].opt())
                    for g in range(8):
                        ps = pp1.tile([128, Nt], fp32)
                        for k in range(8):
                            nc.tensor.matmul(ps[:],
                                             w1sb[:, k, 128 * g:128 * (g + 1)],
                                             rhs[:, k, :],
                                             start=(k == 0), stop=(k == 7))
                        nc.vector.tensor_scalar_add(
                            gx1sb[:, g, b * Nt:(b + 1) * Nt], ps[:],
                            b1sb[:, g:g + 1])

            # ---- R1: H=256 recurrence ----
            if nphases >= 3:
             with (
                tc.tile_pool(name="r1c", bufs=1) as cp,
                tc.tile_pool(name="psi", bufs=2, space="PSUM") as ppi,
                tc.tile_pool(name="psg", bufs=2, space="PSUM") as ppg,
                tc.tile_pool(name="r1w", bufs=2) as wk,
            ):
                lhsT0 = cp.tile([128, 1024], bf16)
                lhsT1 = cp.tile([128, 1024], bf16)
                nc.sync.dma_start(lhsT0[:], l1T[0:128, :])
                nc.sync.dma_start(lhsT1[:], l1T[128:256, :])
                gxc = cp.tile([128, 8, C2], fp32)
                hb = cp.tile([128, 2, C2], bf16)
                c0t = cp.tile([128, 2], fp32)
                c1t = cp.tile([128, 2], fp32)
                cbuf = [c0t, c1t]
                nc.vector.memset(c0t[:], 0.0)
                nc.vector.memset(hb[:, :, C2 - 1:C2], 0.0)
                nc.vector.tensor_copy(gxc[:, :, 0:CH], gx1sb[:, :, 0:CH])

                def step1(s):
                    hcol = (s - 1) % C2
                    psi = ppi.tile([128, 6], fp32)
                    psg = ppg.tile([128, 2], fp32)
                    sifo = wk.tile([128, 6], fp32, tag="sifo")
                    for c in range(6):
                        for k in range(2):
                            lt = (lhsT0, lhsT1)[k]
                            nc.tensor.matmul(psi[:, c:c + 1],
                                             lt[:, 128 * c:128 * (c + 1)],
                                             hb[:, k:k + 1, hcol:hcol + 1],
                                             start=(k == 0), stop=(k == 1))
                    nc.vector.tensor_add(psi[:], psi[:], gxc[:, 0:6, s:s + 1])
                    nc.scalar.activation(sifo[:], psi[:], AF.Sigmoid)
                    for c in range(6, 8):
                        for k in range(2):
                            lt = (lhsT0, lhsT1)[k]
                            nc.tensor.matmul(psg[:, c - 6:c - 5],
                                             lt[:, 128 * c:128 * (c + 1)],
                                             hb[:, k:k + 1, hcol:hcol + 1],
                                             start=(k == 0), stop=(k == 1))
                    nc.vector.tensor_add(psg[:], psg[:], gxc[:, 6:8, s:s + 1])
                    gt = wk.tile([128, 2], fp32, tag="gt")
                    nc.scalar.activation(gt[:], psg[:], AF.Tanh)
                    m1 = wk.tile([128, 2], fp32, tag="m1")
                    nc.vector.tensor_mul(m1[:], sifo[:, 0:2], gt[:])
                    m2 = wk.tile([128, 2], fp32, tag="m2")
                    nc.vector.tensor_mul(m2[:], sifo[:, 2:4], cbuf[s % 2][:])
                    nc.vector.tensor_add(cbuf[(s + 1) % 2][:], m1[:], m2[:])
                    tcc = wk.tile([128, 2], fp32, tag="tcc")
                    nc.scalar.activation(tcc[:], cbuf[(s + 1) % 2][:], AF.Tanh)
                    nc.vector.tensor_mul(hb[:, :, s:s + 1], sifo[:, 4:6],
                                         tcc[:])

                def wr1(i, lo):
                    # write hb[:, :, lo:lo+CH] fwd + reversed
                    for p in range(2):
                        nc.sync.dma_start(y1cb[0, p, :, ds(i + lo, CH)],
                                          hb[:, p, lo:lo + CH])
                        hr = wk.tile([128, CH], bf16, tag=f"hr{p}")
                        nc.vector.tensor_copy(hr[:],
                                              hb[:, p, lo:lo + CH][:, ::-1])
                        nc.gpsimd.dma_start(
                            y1cb[1, p, :, ds(T - CH - lo - i, CH)], hr[:])

                with tc.For_i(0, T, C2, hint_engines=HINTS) as i:
                    nc.vector.tensor_copy(gxc[:, :, CH:C2],
                                          gx1sb[:, :, ds(i + CH, CH)])
                    for s in range(CH):
                        step1(s)
                    nc.vector.tensor_copy(gxc[:, :, 0:CH],
                                          gx1sb[:, :, ds(i + C2, CH)])
                    wr1(i, 0)
                    for s in range(CH, C2):
                        step1(s)
                    wr1(i, CH)

        if nphases >= 3:
            nc.gpsimd.collective_compute(
                "AllGather", mybir.AluOpType.bypass,
                replica_groups=[[0, 1]],
                ins=[y1cb[:].opt()], outs=[Y1G[:].opt()])

        # ================= R2: H=64, embedded gx2 GEMM ==================
        if nphases >= 4:
         with (
            tc.tile_pool(name="r2c", bufs=1) as cp,
            tc.tile_pool(name="r2r", bufs=4) as rp,
            tc.tile_pool(name="psG", bufs=4, space="PSUM") as ppG,
            tc.tile_pool(name="psA", bufs=2, space="PSUM") as ppA,
            tc.tile_pool(name="psB", bufs=2, space="PSUM") as ppB,
            tc.tile_pool(name="r2w", bufs=2) as wk,
        ):
            w2sb = cp.tile([128, 4, 256], bf16)
            l2sb = cp.tile([64, 256], fp32)
            b2sb = cp.tile([128, 2], fp32)
            for k in range(4):
                nc.sync.dma_start(w2sb[:, k, :], w2T[128 * k:128 * (k + 1), :])
            nc.sync.dma_start(l2sb[:], l2T[:])
            nc.sync.dma_start(b2sb[:], b2d[:])
            gxP = [cp.tile([128, T], fp32, tag=f"gx{p}", name=f"gxP{p}")
                   for p in range(2)]
            for b in range(NB):
                rhs = rp.tile([128, 4, Nt], bf16, tag="rhs")
                for k in range(2):
                    nc.sync.dma_start(
                        rhs[:, k, :],
                        Y1G[ds(myid, 1), 0, k, :, b * Nt:(b + 1) * Nt].opt())
                    nc.gpsimd.dma_start(
                        rhs[:, 2 + k, :],
                        Y1G[ds(oth, 1), 1, k, :, b * Nt:(b + 1) * Nt].opt())
                for p in range(2):
                    ps = ppG.tile([128, Nt], fp32)
                    for k in range(4):
                        nc.tensor.matmul(ps[:], w2sb[:, k, 128 * p:128 * (p + 1)],
                                         rhs[:, k, :], start=(k == 0),
                                         stop=(k == 3))
                    nc.vector.tensor_scalar_add(
                        gxP[p][:, b * Nt:(b + 1) * Nt], ps[:], b2sb[:, p:p + 1])
            # ---- recurrence ----
            y2r = cp.tile([64, C2], fp32)
            gxc = cp.tile([128, 2, C2], fp32)
            c0t = cp.tile([64, 1], fp32)
            c1t = cp.tile([64, 1], fp32)
            cbuf = [c0t, c1t]
            nc.vector.memset(c0t[:], 0.0)
            nc.vector.memset(y2r[:, C2 - 1:C2], 0.0)

            def step2(s):
                hcol = (s - 1) % C2
                psA = ppA.tile([128, 1], fp32)
                psB = ppB.tile([128, 1], fp32)
                nc.tensor.matmul(psA[:], l2sb[:, 0:128], y2r[:, hcol:hcol + 1],
                                 start=True, stop=True)
                nc.tensor.matmul(psB[:], l2sb[:, 128:256], y2r[:, hcol:hcol + 1],
                                 start=True, stop=True)
                sc0 = wk.tile([128, 1], fp32, tag="sc0")   # [i'; f']
                nc.scalar.activation(sc0[:], psA[:], AF.Sigmoid,
                                     bias=gxc[:, 0:1, s:s + 1])
                gt = wk.tile([64, 1], fp32, tag="gt")
                nc.scalar.activation(gt[:], psB[64:128, :], AF.Tanh,
                                     bias=gxc[64:128, 1:2, s:s + 1])
                so = wk.tile([64, 1], fp32, tag="so")
                nc.scalar.activation(so[:], psB[0:64, :], AF.Sigmoid,
                                     bias=gxc[0:64, 1:2, s:s + 1])
                m1 = wk.tile([64, 1], fp32, tag="m1")
                nc.vector.tensor_mul(m1[:], sc0[0:64, :], gt[:])
                tcc = wk.tile([64, 1], fp32, tag="tcc")
                nc.scalar.activation(tcc[:], cbuf[s % 2][:], AF.Tanh,
                                     bias=m1[:], scale=sc0[64:128, :])
                nc.scalar.activation(y2r[:, s:s + 1], tcc[:], AF.Copy,
                                     scale=so[:])
                nc.vector.tensor_scalar(cbuf[(s + 1) % 2][:],
                                        cbuf[s % 2][:], sc0[64:128, :],
                                        m1[:], ALU.mult, ALU.add)

            def wr2(i, lo):
                y16 = wk.tile([64, CH], bf16, tag="y16")
                nc.vector.tensor_copy(y16[:], y2r[:, lo:lo + CH])
                nc.sync.dma_start(y2cb[0, :, ds(i + lo, CH)], y16[:])
                y16r = wk.tile([64, CH], bf16, tag="y16r")
                nc.vector.tensor_copy(y16r[:], y2r[:, lo:lo + CH][:, ::-1])
                nc.gpsimd.dma_start(y2cb[1, :, ds(T - CH - lo - i, CH)], y16r[:])

            with tc.For_i(0, T, C2, hint_engines=HINTS) as i:
                nc.vector.tensor_copy(gxc[:, 0:1, :], gxP[0][:, ds(i, C2)])
                nc.vector.tensor_copy(gxc[:, 1:2, :], gxP[1][:, ds(i, C2)])
                for s in range(CH):
                    step2(s)
                wr2(i, 0)
                for s in range(CH, C2):
                    step2(s)
                wr2(i, CH)

        if nphases >= 4:
            nc.gpsimd.collective_compute(
                "AllGather", mybir.AluOpType.bypass,
                replica_groups=[[0, 1]],
                ins=[y2cb[:].opt()], outs=[Y2G[:].opt()])

        # ================= R3: H=32, embedded gx3 GEMM ==================
        if nphases >= 5:
         with (
            tc.tile_pool(name="r3c", bufs=1) as cp,
            tc.tile_pool(name="r3r", bufs=4) as rp,
            tc.tile_pool(name="ps3G", bufs=4, space="PSUM") as ppG,
            tc.tile_pool(name="ps3R", bufs=2, space="PSUM") as ppR,
            tc.tile_pool(name="r3w", bufs=2) as wk,
        ):
            w3sb = cp.tile([128, 128], bf16)
            l3sb = cp.tile([32, 128], fp32)
            b3sb = cp.tile([128, 1], fp32)
            nc.sync.dma_start(w3sb[:], w3T[:])
            nc.sync.dma_start(l3sb[:], l3T[:])
            nc.sync.dma_start(b3sb[:], b3d[:])
            gx3 = cp.tile([128, T], fp32)
            for b in range(NB):
                rhs = rp.tile([128, Nt], bf16, tag="rhs")
                nc.sync.dma_start(
                    rhs[0:64, :],
                    Y2G[ds(myid, 1), 0, :, b * Nt:(b + 1) * Nt].opt())
                nc.gpsimd.dma_start(
                    rhs[64:128, :],
                    Y2G[ds(oth, 1), 1, :, b * Nt:(b + 1) * Nt].opt())
                ps = ppG.tile([128, Nt], fp32)
                nc.tensor.matmul(ps[:], w3sb[:], rhs[:], start=True, stop=True)
                nc.vector.tensor_scalar_add(gx3[:, b * Nt:(b + 1) * Nt],
                                            ps[:], b3sb[:, 0:1])
            h3 = cp.tile([32, 1], fp32)
            gxc = cp.tile([128, C2], fp32)
            c0t = cp.tile([32, 1], fp32)
            c1t = cp.tile([32, 1], fp32)
            cbuf = [c0t, c1t]
            nc.vector.memset(c0t[:], 0.0)
            nc.vector.memset(h3[:], 0.0)

            def step3(s):
                ps = ppR.tile([128, 1], fp32)
                nc.tensor.matmul(ps[:], l3sb[:], h3[:], start=True, stop=True)
                sifo = wk.tile([96, 1], fp32, tag="sifo")
                nc.scalar.activation(sifo[:], ps[0:96, :], AF.Sigmoid,
                                     bias=gxc[0:96, s:s + 1])
                gt = wk.tile([32, 1], fp32, tag="gt")
                nc.scalar.activation(gt[:], ps[96:128, :], AF.Tanh,
                                     bias=gxc[96:128, s:s + 1])
                m1 = wk.tile([32, 1], fp32, tag="m1")
                nc.scalar.activation(m1[:], gt[:], AF.Copy,
                                     scale=sifo[0:32, :])
                tcc = wk.tile([32, 1], fp32, tag="tcc")
                nc.scalar.activation(tcc[:], cbuf[s % 2][:], AF.Tanh,
                                     bias=m1[:], scale=sifo[32:64, :])
                nc.scalar.activation(h3[:], tcc[:], AF.Copy,
                                     scale=sifo[64:96, :])
                nc.vector.tensor_scalar(cbuf[(s + 1) % 2][:],
                                        cbuf[s % 2][:], sifo[32:64, :],
                                        m1[:], ALU.mult, ALU.add)

            with tc.For_i(0, T, C2, hint_engines=HINTS) as i:
                nc.vector.tensor_copy(gxc[:], gx3[:, ds(i, C2)])
                for s in range(C2):
                    step3(s)
            nc.sync.dma_start(hout[:], h3[:])
    nc.compile()
    return nc


# --------------------------------------------------------------------------
# Host-side prep + launch
# --------------------------------------------------------------------------
def perm_ifog(H):
    """pytorch gate rows [i,f,g,o] -> [i,f,o,g]"""
    return np.r_[0:2 * H, 3 * H:4 * H, 2 * H:3 * H]


def _c32(a):
    return np.ascontiguousarray(a, dtype=np.float32)


def _c16(a):
    return np.ascontiguousarray(np.asarray(a).astype(ml_dtypes.bfloat16))


def _cx(a):
    return np.ascontiguousarray(np.asarray(a).astype(getattr(ml_dtypes, XDT_NP)))


class Pipeline:
    def __init__(self, nphases=5, **kw):
        self.nc = build_fused(nphases)

    def __call__(self, inputs, timings=None):
        import time as _time
        ii = {k: np.asarray(v) for k, v in inputs.items()}
        p1, p2, p3 = perm_ifog(256), perm_ifog(64), perm_ifog(32)
        xx = ii["x"].astype(getattr(ml_dtypes, XDT_NP))   # [8192, 1024]
        maps = []
        for c, d in ((0, "f"), (1, "b")):
            b1 = (ii[f"l1{d}_bih"] + ii[f"l1{d}_bhh"])[p1]
            b2 = (ii[f"l2{d}_bih"] + ii[f"l2{d}_bhh"])[p2]
            b3 = (ii[f"l3{d}_bih"] + ii[f"l3{d}_bhh"])[p3]
            # The rhs in the layer-2/3 GEMMs is [own-direction stream;
            # other-direction stream].  For the backward core that order is
            # (y_b, y_f), so swap the input-row halves of its W_ih.
            w2Trows = ii[f"l2{d}_wih"][p2].T     # [512, 256]
            w3Trows = ii[f"l3{d}_wih"][p3].T     # [128, 128]
            if c == 1:
                w2Trows = np.concatenate([w2Trows[256:], w2Trows[:256]], 0)
                w3Trows = np.concatenate([w3Trows[64:], w3Trows[:64]], 0)
            maps.append({
                "xTh": np.ascontiguousarray(xx[c * Th:(c + 1) * Th].T),
                "w1T": _cx(ii[f"l1{d}_wih"][p1].T),
                "b1d": _c32(b1.reshape(8, 128).T),
                "l1T": _c16(ii[f"l1{d}_whh"][p1].T),
                "w2T": _c16(w2Trows),
                "b2d": _c32(b2.reshape(2, 128).T),
                "l2T": _c32(ii[f"l2{d}_whh"][p2].T),
                "w3T": _c16(w3Trows),
                "b3d": _c32(b3.reshape(128, 1)),
                "l3T": _c32(ii[f"l3{d}_whh"][p3].T),
            })
        self._last_maps = maps
        t0 = _time.time()
        res = run_bass_kernel_spmd(self.nc, maps, [0, 1]).results
        if timings is not None:
            timings["fused"] = _time.time() - t0
        h3f = res[0]["hout"][:, 0]
        h3b = res[1]["hout"][:, 0]
        feat = np.concatenate([h3f, h3b])[None, :].astype(np.float32)
        z = feat @ ii["w1"].T + ii["b1"]
        z = z @ ii["w2"].T + ii["b2"]
        return z.astype(np.float32)


# --------------------------------------------------------------------------
# harness entry point
# --------------------------------------------------------------------------
_PIPE = None


def kernel(**inputs):
    global _PIPE
    if _PIPE is None:
        _PIPE = Pipeline()
    inp = {k: np.asarray(v) for k, v in inputs.items()}
    return _PIPE(inp)
